# revision 7
# baseline (speedup 1.0000x reference)
"""Trainium2 Bass kernel for DeepProteinClassifier.

Contract: kernel(**inputs) takes the FULL unsharded inputs and returns
the FULL [32, 10] float32 output.

Sharding: data-parallel over batch B=32 across 8 NeuronCores (4 samples
per core); all weights replicated.

Optimizations over the naive formulation:
- Mask compaction + per-slot sizing: only mask==1 positions matter.
  Samples are sorted by kept-count and dealt into 4 slots of 8 (one per
  core), so slot j's compiled budget is the max count of its group:
  (SPQ, NK) per slot, e.g. (544,5),(520,5),(520,5),(504,4) -- the
  smallest slot runs last, shrinking both PE work and the endgame tail.
  The program is compiled per budget tuple (cached).
- Fused QK: scores = x M x^T + key-bias with M = Wq^T Wk / sqrt(D);
  per-query constants cancel in softmax; per-key term is a host bias
  folded into the exp bias. Deletes one 960x960 projection.
- fp8 (e4m3) matmuls in DoubleRow mode, fp32 PSUM accumulation.
- LayerNorm stats via the DVE bn_stats/bn_aggr hardware path (two
  equal 480-wide chunks so the aggregate combine is exact); residual
  adds fused into the PSUM drains (scalar_tensor_tensor); the second
  residual half runs on the otherwise-idle GPSIMD engine.
- Masked-mean pool as PE matvec with a zero-padded 4-column lhsT so
  sample j lands on PSUM partition j; all 4 samples accumulate into one
  persistent PSUM pair, drained once before the MLP transposes (no
  SBUF->SBUF DMAs).
- PE warm-up: dummy matmuls issued during the initial DMA wait flip the
  HAM clock gate to 2.4 GHz before real work; attention weights are
  fetched on the GpSimd queue so the ACT table load can't delay them.
"""

import numpy as np
import ml_dtypes

B, S, D = 32, 1024, 960
NCORES = 8
BPC = B // NCORES   # 4 samples (slots) per core
SPMAX = 640         # kept positions hard cap (5 tiles of 128)
DP = 1024           # padded contraction dim (8 chunks of 128, 4 DR pairs)
NDC = 8             # number of 128-row d chunks
PD = 120            # MLP-side partition size (960 = 8*120)
LN_EPS = 1e-5
SC_M = 1024.0       # host scale on M (undone in exp drain)
SC_V = 16.0         # host scale on Wv (undone in V drain)
BF16 = ml_dtypes.bfloat16
F8 = ml_dtypes.float8_e4m3

_CACHE = {}


def _build_nc(cfg):
    """cfg: tuple of (spq, nk) per slot j=0..3, spq multiple of 8."""
    import concourse.tile as tile
    from concourse import bacc, mybir

    class _Bacc(bacc.Bacc):
        """Bacc with the ACT table chooser steered to the combined
        ln+exp function set so LayerNorm's Ln/Exp pair and the softmax
        exp share ONE table (no ACT_TABLE_LOAD thrash)."""

        def insert_act_table_loads(self):
            import bass_rust as _bass_rust
            from concourse.hw_specs import get_activation_tables

            has_activation = any(
                isinstance(i, mybir.InstActivation)
                for b in self.main_func.blocks
                for i in b.instructions
            )
            if not has_activation:
                return
            tables = list(get_activation_tables(self.m.arch).items())
            combo = next(
                (f for n, f in tables if n == "natural_log_exp_and_others"), None
            )
            if combo is not None:
                tables = [
                    (n, f if n == "natural_log_exp_and_others" else f - combo)
                    for n, f in tables
                ]
            _bass_rust.insert_act_table_loads(self, tables)

    f32 = mybir.dt.float32
    bf16 = mybir.dt.bfloat16
    f8 = mybir.dt.float8e4
    Alu = mybir.AluOpType
    Act = mybir.ActivationFunctionType
    DR = mybir.MatmulPerfMode.DoubleRow

    nc = _Bacc("TRN2", target_bir_lowering=False, debug=False)

    # T1T tile width: covers the largest slot's q budget, 16B-aligned
    # (DoubleRow rhs strides must be multiples of 16)
    spq_alloc = (max(c[0] for c in cfg) + 15) // 16 * 16

    # ---- DRAM parameters (per-core shard) ----
    xt_h = nc.declare_dram_parameter("xt", [BPC, DP, SPMAX], f8, isOutput=False)
    xn_h = nc.declare_dram_parameter("xn", [BPC, SPMAX, D], bf16, isOutput=False)
    mnp_h = nc.declare_dram_parameter("mnp", [BPC, 128, 5], f32, isOutput=False)
    mfs_h = nc.declare_dram_parameter("mfs", [BPC, 128, 5], f32, isOutput=False)
    m8_h = nc.declare_dram_parameter("m8", [DP, DP], f8, isOutput=False)
    wv_h = nc.declare_dram_parameter("wv", [DP, D], f8, isOutput=False)
    w1s_h = nc.declare_dram_parameter("w1s", [1, 512], bf16, isOutput=False)
    w1_h = nc.declare_dram_parameter("w1", [D, 512], bf16, isOutput=False)
    w2_h = nc.declare_dram_parameter("w2", [512, 256], bf16, isOutput=False)
    w3_h = nc.declare_dram_parameter("w3", [256, 128], bf16, isOutput=False)
    w4_h = nc.declare_dram_parameter("w4", [128, 10], bf16, isOutput=False)
    b1_h = nc.declare_dram_parameter("b1", [128, 4], f32, isOutput=False)
    b2_h = nc.declare_dram_parameter("b2", [128, 2], f32, isOutput=False)
    b3_h = nc.declare_dram_parameter("b3", [128, 1], f32, isOutput=False)
    b4_h = nc.declare_dram_parameter("b4", [10, 1], f32, isOutput=False)
    id4_h = nc.declare_dram_parameter("id4", [4, 4], f32, isOutput=False)
    out_h = nc.declare_dram_parameter("out", [10, BPC], f32, isOutput=True)

    with tile.TileContext(nc) as tc:
        with (
            tc.tile_pool(name="wpool", bufs=1) as wpool,
            tc.tile_pool(name="xpool", bufs=3) as xpool,
            tc.tile_pool(name="big", bufs=2) as big,
            tc.tile_pool(name="stats", bufs=3) as stats,
            tc.tile_pool(name="psum", bufs=6, space="PSUM") as psum,
            tc.tile_pool(name="ppool", bufs=1, space="PSUM") as ppool,
        ):
            # ---- PE warm-up: flip the HAM clock gate to 8/8 during the
            #      initial DMA wait (dummy matmuls on a memset scratch) --
            wscr = wpool.tile([128, 512], f8, name="wscr")
            nc.vector.memset(wscr[:], 0.25)
            psw = psum.tile([128, 512], f32, tag="mm", name="psw")
            for i in range(8):
                nc.tensor.matmul(
                    psw[:], lhsT=wscr[:, 0:128], rhs=wscr[:],
                    start=(i == 0), stop=(i == 7),
                )
            wsink = wpool.tile([1, 1], f32, name="wsink")
            nc.vector.tensor_copy(wsink[:], psw[0:1, 0:1])
            epsc = wpool.tile([128, 1], f32, name="epsc")
            nc.vector.memset(epsc[:], LN_EPS)

            def load_sample(j, defer=False):
                nk = cfg[j][1]
                kc = nk * 128
                xt_sb = xpool.tile([128, NDC, SPMAX], f8, tag="xt", name=f"xt{j}")
                if defer:
                    # pair-granular so the first T1T matmuls start early
                    for p in range(4):
                        nc.sync.dma_start(
                            xt_sb[:, 2 * p : 2 * p + 2, 0:kc],
                            xt_h[j, 256 * p : 256 * (p + 1), 0:kc].rearrange(
                                "(c p) s -> p c s", p=128
                            ),
                        )
                else:
                    nc.sync.dma_start(
                        xt_sb[:, :, 0:kc],
                        xt_h[j, :, 0:kc].rearrange("(c p) s -> p c s", p=128),
                    )
                xn_sb = xpool.tile([128, 5, D], bf16, tag="xn", name=f"xn{j}")
                mnp_sb = stats.tile([128, 5], f32, tag="mnp", name=f"mnp{j}")
                mfs_sb = stats.tile([128, 5], f32, tag="mfs", name=f"mfs{j}")
                if not defer:
                    nc.sync.dma_start(
                        xn_sb[:, 0:nk, :],
                        xn_h[j, 0:kc].rearrange("(t p) d -> p t d", p=128),
                    )
                    nc.sync.dma_start(mnp_sb[:], mnp_h[j])
                    nc.sync.dma_start(mfs_sb[:], mfs_h[j])
                return xt_sb, xn_sb, mnp_sb, mfs_sb

            # xt0 + attention weights first (they gate the first matmuls).
            # Weights go on the GpSimd HWDGE queue: the scalar queue's
            # ACT_TABLE_LOAD would delay their ring kick by ~1.3us.
            sample0 = load_sample(0, defer=True)
            m8_sb = wpool.tile([128, NDC, DP], f8)
            wv_sb = wpool.tile([128, NDC, DP], f8)
            nc.gpsimd.dma_start(
                m8_sb[:, :, 0:256],
                m8_h[:, 0:256].rearrange("(c p) n -> p c n", p=128),
            )
            nc.gpsimd.dma_start(
                m8_sb[:, :, 256:512],
                m8_h[:, 256:512].rearrange("(c p) n -> p c n", p=128),
            )
            nc.gpsimd.dma_start(
                wv_sb[:, :, 0:512],
                wv_h[:, 0:512].rearrange("(c p) n -> p c n", p=128),
            )
            nc.gpsimd.dma_start(
                m8_sb[:, :, 512:DP],
                m8_h[:, 512:DP].rearrange("(c p) n -> p c n", p=128),
            )
            nc.gpsimd.dma_start(
                wv_sb[:, :, 512:D],
                wv_h[:, 512:D].rearrange("(c p) n -> p c n", p=128),
            )
            nk0 = cfg[0][1]
            nc.sync.dma_start(
                sample0[1][:, 0:nk0, :],
                xn_h[0, 0 : nk0 * 128].rearrange("(t p) d -> p t d", p=128),
            )
            nc.sync.dma_start(sample0[2][:], mnp_h[0])
            nc.sync.dma_start(sample0[3][:], mfs_h[0])

            pooled_sb = wpool.tile([BPC, D + 1], f32, name="pooled_sb")
            murow = wpool.tile([1, BPC], bf16)
            mlp_w = {}
            # persistent pool accumulators: sample j lands on partition j
            pp0 = ppool.tile([BPC, 512], f32, name="pp0")
            pp1 = ppool.tile([BPC, 449], f32, name="pp1")

            def load_mlp_weights():
                w1s_sb = wpool.tile([1, 512], bf16, name="w1s_sb")
                nc.sync.dma_start(w1s_sb[:], w1s_h[:])
                mlp_w["w1s"] = w1s_sb
                w1_sb = wpool.tile([PD, NDC, 512], bf16, name="w1_sb")
                nc.sync.dma_start(w1_sb[:], w1_h[:].rearrange("(c p) n -> p c n", p=PD))
                w2_sb = wpool.tile([128, 4, 256], bf16, name="w2_sb")
                nc.sync.dma_start(w2_sb[:], w2_h[:].rearrange("(c p) n -> p c n", p=128))
                w3_sb = wpool.tile([128, 2, 128], bf16, name="w3_sb")
                nc.sync.dma_start(w3_sb[:], w3_h[:].rearrange("(c p) n -> p c n", p=128))
                w4_sb = wpool.tile([128, 10], bf16, name="w4_sb")
                nc.sync.dma_start(w4_sb[:], w4_h[:])
                b1_sb = wpool.tile([128, 4], f32, name="b1_sb")
                nc.sync.dma_start(b1_sb[:], b1_h[:])
                b2_sb = wpool.tile([128, 2], f32, name="b2_sb")
                nc.sync.dma_start(b2_sb[:], b2_h[:])
                b3_sb = wpool.tile([128, 1], f32, name="b3_sb")
                nc.sync.dma_start(b3_sb[:], b3_h[:])
                b4_sb = wpool.tile([10, 1], f32, name="b4_sb")
                nc.sync.dma_start(b4_sb[:], b4_h[:])
                id4_sb = wpool.tile([4, 4], f32, name="id4_sb")
                nc.sync.dma_start(id4_sb[:], id4_h[:])
                mlp_w.update(w1=w1_sb, w2=w2_sb, w3=w3_sb, w4=w4_sb,
                             b1=b1_sb, b2=b2_sb, b3=b3_sb, b4=b4_sb, id4=id4_sb)

            pending_pool = None

            for j in range(BPC):
                spq, nk = cfg[j]
                w0 = min(512, spq)      # main q stream width
                tw = spq - w0           # tail q width (may be 0)
                # q-tile widths (partial last tile)
                qws = [min(128, spq - qt * 128) for qt in range(nk)]
                qws = [w for w in qws if w > 0]
                qt_n = len(qws)

                if j == 0:
                    xt_sb, xn_sb, mnp_sb, mfs_sb = sample0
                else:
                    xt_sb, xn_sb, mnp_sb, mfs_sb = load_sample(j)
                if j == 1:
                    load_mlp_weights()

                # ---- T1T = M^T-chunks @ xT-chunks: [do(1024), q(spq)] fp8 --
                T1T = big.tile([128, NDC, spq_alloc], f8, tag="T1T", name=f"T1T{j}")
                V = big.tile([128, 5, 1024], f8, tag="V", name=f"V{j}")
                nc.vector.memset(V[:, 0:nk, D : D + 1], 1.0)
                if tw:
                    psTail = psum.tile([128, NDC, tw], f32, tag="mm",
                                       name=f"ptail{j}")

                def t1_chunks(ts, te):
                  with nc.named_scope(f"s{j}_t1"):
                    for t in range(ts, te):
                        psA = psum.tile([128, w0], f32, tag="mm", name="pt1a")
                        for p in range(4):
                            lw = m8_sb[:, 2 * p : 2 * p + 2, t * 128 : (t + 1) * 128]
                            nc.tensor.matmul(
                                psA[:], lhsT=lw,
                                rhs=xt_sb[:, 2 * p : 2 * p + 2, 0:w0],
                                start=(p == 0), stop=(p == 3), perf_mode=DR,
                            )
                            if tw:
                                nc.tensor.matmul(
                                    psTail[:, t, :], lhsT=lw,
                                    rhs=xt_sb[:, 2 * p : 2 * p + 2, 512:spq],
                                    start=(p == 0), stop=(p == 3), perf_mode=DR,
                                )
                        nc.vector.tensor_copy(T1T[:, t, 0:w0], psA[:])
                        if tw:
                            nc.vector.tensor_copy(
                                T1T[:, t, 512:spq], psTail[:, t, :]
                            )

                def v_half(lo, hi):
                  with nc.named_scope(f"s{j}_v"):
                    for st in range(nk):
                        ps = psum.tile([128, 512], f32, tag="mm", name="psv")
                        for p in range(4):
                            lx = xt_sb[:, 2 * p : 2 * p + 2, st * 128 : (st + 1) * 128]
                            nc.tensor.matmul(
                                ps[:, 0 : hi - lo], lhsT=lx,
                                rhs=wv_sb[:, 2 * p : 2 * p + 2, lo:hi],
                                start=(p == 0), stop=(p == 3), perf_mode=DR,
                            )
                        if lo == 0:
                            nc.scalar.activation(
                                V[:, st, lo:hi], ps[:, 0 : hi - lo],
                                Act.Copy, scale=1.0 / SC_V,
                            )
                        else:
                            nc.vector.tensor_scalar_mul(
                                V[:, st, lo:hi], ps[:, 0 : hi - lo], 1.0 / SC_V
                            )

                # T1T t0-3 needs only the first m8 half; V's first half then
                # runs while the later weight-DMA halves land
                t1_chunks(0, 4)
                v_half(0, 512)
                t1_chunks(4, NDC)
                v_half(512, D)

                # ---- ST = xT^T @ T1T; ET = exp(ST/1024 + keybias) fp8 ----
                ET = big.tile([128, 5, SPMAX], f8, tag="ET", name=f"ET{j}")
                if tw:
                    psTailS = psum.tile([128, 5, tw], f32, tag="mm",
                                        name=f"stail{j}")
                with nc.named_scope(f"s{j}_st"):
                    for kt in range(nk):
                        psA = psum.tile([128, w0], f32, tag="mm", name="pssa")
                        for p in range(4):
                            lx = xt_sb[:, 2 * p : 2 * p + 2, kt * 128 : (kt + 1) * 128]
                            nc.tensor.matmul(
                                psA[:], lhsT=lx,
                                rhs=T1T[:, 2 * p : 2 * p + 2, 0:w0],
                                start=(p == 0), stop=(p == 3), perf_mode=DR,
                            )
                            if tw:
                                nc.tensor.matmul(
                                    psTailS[:, kt, :], lhsT=lx,
                                    rhs=T1T[:, 2 * p : 2 * p + 2, 512:spq],
                                    start=(p == 0), stop=(p == 3), perf_mode=DR,
                                )
                        nc.scalar.activation(
                            ET[:, kt, 0:w0], psA[:], Act.Exp,
                            bias=mnp_sb[:, kt : kt + 1], scale=1.0 / SC_M,
                        )
                        if tw:
                            nc.scalar.activation(
                                ET[:, kt, 512:spq], psTailS[:, kt, :], Act.Exp,
                                bias=mnp_sb[:, kt : kt + 1], scale=1.0 / SC_M,
                            )

                # previous sample's pool matvec lands here: its AL/H are long
                # ready, and it fills the PE while the ET exp drains finish
                if pending_pool is not None:
                    pending_pool()
                    pending_pool = None

                # ---- context + residual + per-tile LN stats --------------
                # LayerNorm is per-row: tile qt's alpha is ready as soon as
                # its context drains, so the pool matvec pipelines per-tile.
                H = big.tile([128, 5, 1024], bf16, tag="H", name=f"H{j}")
                recips = stats.tile([128, 5], f32, tag="recips", name=f"rc{j}")
                bn = stats.tile([128, 5, 12], f32, tag="bn", name=f"bn{j}")
                MV = stats.tile([128, 5, 2], f32, tag="MV", name=f"MV{j}")
                lnv = stats.tile([128, 5], f32, tag="lnv", name=f"lnv{j}")
                rs = stats.tile([128, 5], f32, tag="rs", name=f"rs{j}")
                AL4 = stats.tile([128, 5, BPC], bf16, tag="AL4", name=f"AL{j}")
                nc.vector.memset(AL4[:], 0.0)
                with nc.named_scope(f"s{j}_ctx"):
                    for qt in range(qt_n):
                        pw = qws[qt]
                        qo = qt * 128
                        ps0 = psum.tile([128, 512], f32, tag="mm", name="psc0")
                        ps1 = psum.tile([128, 449], f32, tag="mm", name="psc1")
                        for p in range(nk // 2):
                            le = ET[:, 2 * p : 2 * p + 2, qo : qo + pw]
                            nc.tensor.matmul(
                                ps0[:pw, :], lhsT=le,
                                rhs=V[:, 2 * p : 2 * p + 2, 0:512],
                                start=(p == 0), stop=(nk % 2 == 0 and p == nk // 2 - 1),
                                perf_mode=DR,
                            )
                            nc.tensor.matmul(
                                ps1[:pw, :], lhsT=le,
                                rhs=V[:, 2 * p : 2 * p + 2, 512 : D + 1],
                                start=(p == 0), stop=(nk % 2 == 0 and p == nk // 2 - 1),
                                perf_mode=DR,
                            )
                        if nk % 2:
                            le = ET[:, nk - 1, qo : qo + pw]
                            nc.tensor.matmul(
                                ps0[:pw, :], lhsT=le, rhs=V[:, nk - 1, 0:512],
                                start=False, stop=True,
                            )
                            nc.tensor.matmul(
                                ps1[:pw, :], lhsT=le, rhs=V[:, nk - 1, 512 : D + 1],
                                start=False, stop=True,
                            )
                        q = slice(qt, qt + 1)
                        # col 448 of ps1: softmax denom (V ones column)
                        nc.vector.reciprocal(recips[:pw, q], ps1[:pw, 448:449])
                        # H = ctx/r + xn, fused PSUM drain + residual
                        nc.vector.scalar_tensor_tensor(
                            H[:pw, qt, 0:512], ps0[:pw, :], recips[:pw, q],
                            xn_sb[:pw, qt, 0:512], Alu.mult, Alu.add,
                        )
                        cscr = stats.tile([128, 448], bf16, tag="cscr",
                                          name=f"cs{j}_{qt}", bufs=2)
                        nc.scalar.activation(
                            cscr[:pw, :], ps1[:pw, 0:448], Act.Copy,
                            scale=recips[:pw, q],
                        )
                        nc.gpsimd.tensor_add(
                            H[:pw, qt, 512:D], cscr[:pw, :],
                            xn_sb[:pw, qt, 512:D],
                        )
                        # LN stats via bn_stats/bn_aggr (2 equal 480 chunks)
                        nc.vector.bn_stats(bn[:pw, qt, 0:6], H[:pw, qt, 0:480])
                        nc.vector.bn_stats(bn[:pw, qt, 6:12], H[:pw, qt, 480:D])
                        nc.vector.bn_aggr(MV[:pw, qt, :], bn[:pw, qt, :])
                        # mean -> H col 960 (pooled into murow later)
                        nc.vector.tensor_copy(
                            H[:pw, qt, D : D + 1], MV[:pw, qt, 0:1]
                        )
                        # rs = exp(-0.5 ln(var+eps)); AL = mfs * rs
                        nc.scalar.activation(
                            lnv[:pw, q], MV[:pw, qt, 1:2], Act.Ln,
                            bias=epsc[:pw, :],
                        )
                        nc.scalar.activation(
                            rs[:pw, q], lnv[:pw, q], Act.Exp, scale=-0.5
                        )
                        nc.vector.tensor_tensor(
                            AL4[:pw, qt, j : j + 1], mfs_sb[:pw, q],
                            rs[:pw, q], Alu.mult,
                        )

                # ---- masked-mean pool as PE matvec into the persistent
                #      PSUM pair; sample j's AL sits in lhsT column j so its
                #      row lands on PSUM partition j. Chunk c waits only on
                #      its own AL column so it pipelines with the LN chain.
                def emit_pool(j=j, AL4=AL4, H=H, qws=qws):
                    for c, pw in enumerate(qws):
                        nc.tensor.matmul(
                            pp0[:, :],
                            lhsT=AL4[:pw, c, :],
                            rhs=H[:pw, c, 0:512],
                            start=(j == 0 and c == 0),
                            stop=(j == BPC - 1 and c == len(qws) - 1),
                        )
                        nc.tensor.matmul(
                            pp1[:, :],
                            lhsT=AL4[:pw, c, :],
                            rhs=H[:pw, c, 512 : D + 1],
                            start=(j == 0 and c == 0),
                            stop=(j == BPC - 1 and c == len(qws) - 1),
                        )

                if j == BPC - 1:
                    # last sample: no next-sample matmuls to hide behind --
                    # emit inline so pool chunks interleave with the LN chain
                    emit_pool()
                else:
                    pending_pool = emit_pool

            # ---- single pool drain + transposes (mu correction is folded
            #      into the W1 matmul as a rank-1 term, see w1s) ----
            nc.vector.tensor_copy(pooled_sb[:, 0:512], pp0[:, :])
            nc.scalar.activation(pooled_sb[:, 512 : D + 1], pp1[:, :], Act.Copy)

            pooledT = stats.tile([PD, NDC, BPC], bf16, tag="pT")
            for c in range(NDC):
                pst = psum.tile([128, 512], f32, tag="mm", name=f"pst{c}")
                nc.tensor.transpose(
                    pst[:PD, :BPC],
                    pooled_sb[:, c * PD : (c + 1) * PD],
                    mlp_w["id4"][:],
                )
                nc.scalar.activation(pooledT[:, c, :], pst[:PD, :BPC], Act.Copy)
            psmu = psum.tile([128, 512], f32, tag="mm", name="psmu")
            nc.tensor.transpose(psmu[:1, :BPC], pooled_sb[:, D : D + 1], mlp_w["id4"][:])
            nc.scalar.activation(murow[:, :], psmu[:1, :BPC], Act.Copy)

            # ---- MLP in transposed layout ----
            h1T = stats.tile([128, 4, BPC], bf16, tag="h1T")
            for m in range(4):
                ps = psum.tile([128, 512], f32, tag="mm", name=f"psm1{m}")
                for c in range(NDC):
                    nc.tensor.matmul(
                        ps[:, :BPC],
                        lhsT=mlp_w["w1"][:, c, m * 128 : (m + 1) * 128],
                        rhs=pooledT[:, c, :],
                        start=(c == 0), stop=False,
                    )
                # rank-1 mu correction: h1 += (-W1e @ ones) * mu
                nc.tensor.matmul(
                    ps[:, :BPC],
                    lhsT=mlp_w["w1s"][:, m * 128 : (m + 1) * 128],
                    rhs=murow[:, :],
                    start=False, stop=True,
                )
                nc.scalar.activation(
                    h1T[:, m, :], ps[:, :BPC], Act.Relu, bias=mlp_w["b1"][:, m : m + 1]
                )
            h2T = stats.tile([128, 2, BPC], bf16, tag="h2T")
            for m in range(2):
                ps = psum.tile([128, 512], f32, tag="mm", name=f"psm2{m}")
                for c in range(4):
                    nc.tensor.matmul(
                        ps[:, :BPC],
                        lhsT=mlp_w["w2"][:, c, m * 128 : (m + 1) * 128],
                        rhs=h1T[:, c, :],
                        start=(c == 0), stop=(c == 3),
                    )
                nc.scalar.activation(
                    h2T[:, m, :], ps[:, :BPC], Act.Relu, bias=mlp_w["b2"][:, m : m + 1]
                )
            h3T = stats.tile([128, 1, BPC], bf16, tag="h3T")
            ps = psum.tile([128, 512], f32, tag="mm", name="psm3")
            for c in range(2):
                nc.tensor.matmul(
                    ps[:, :BPC],
                    lhsT=mlp_w["w3"][:, c, :],
                    rhs=h2T[:, c, :],
                    start=(c == 0), stop=(c == 1),
                )
            nc.scalar.activation(
                h3T[:, 0, :], ps[:, :BPC], Act.Relu, bias=mlp_w["b3"][:, 0:1]
            )
            ps4 = psum.tile([128, 512], f32, tag="mm", name="psm4")
            nc.tensor.matmul(
                ps4[:10, :BPC], lhsT=mlp_w["w4"][:, :], rhs=h3T[:, 0, :],
                start=True, stop=True,
            )
            osb = stats.tile([10, BPC], f32, tag="osb")
            nc.scalar.activation(osb[:], ps4[:10, :BPC], Act.Identity, bias=mlp_w["b4"][:])
            nc.sync.dma_start(out_h[:], osb[:])

    nc.compile()
    return nc


def _get_nc(cfg):
    if cfg not in _CACHE:
        _CACHE[cfg] = _build_nc(cfg)
    return _CACHE[cfg]


def _plan(mask):
    """Sort samples by kept-count (desc), deal into 4 slots of 8 cores.

    Returns (perm, cfg): perm[8*j + c] = original sample index assigned
    to core c slot j; cfg[j] = (spq, nk) compile-time budget of slot j.
    """
    n = mask.sum(axis=1).astype(np.int64)
    perm = np.argsort(-n, kind="stable")
    cfg = []
    for j in range(BPC):
        nmax = int(n[perm[8 * j]])
        nmax = max(nmax, 8)
        assert nmax <= SPMAX, f"slot {j}: {nmax} kept positions > {SPMAX}"
        nk = (nmax + 127) // 128
        spq = min((nmax + 7) // 8 * 8, nk * 128)
        cfg.append((spq, nk))
    return perm, tuple(cfg)


def host_prep(inputs):
    """Build the 8 per-core in_maps from the full inputs."""
    x = np.asarray(inputs["x"], np.float32)
    mask = np.asarray(inputs["mask"])
    Wq, bq = np.asarray(inputs["Wq"], np.float32), np.asarray(inputs["bq"], np.float32)
    Wk = np.asarray(inputs["Wk"], np.float32)
    Wv, bv = np.asarray(inputs["Wv"], np.float32), np.asarray(inputs["bv"], np.float32)
    ln_g, ln_b = np.asarray(inputs["ln_g"], np.float32), np.asarray(inputs["ln_b"], np.float32)
    W1, b1 = np.asarray(inputs["W1"], np.float32), np.asarray(inputs["b1"], np.float32)
    W2, b2 = np.asarray(inputs["W2"], np.float32), np.asarray(inputs["b2"], np.float32)
    W3, b3 = np.asarray(inputs["W3"], np.float32), np.asarray(inputs["b3"], np.float32)
    W4, b4 = np.asarray(inputs["W4"], np.float32), np.asarray(inputs["b4"], np.float32)

    perm, cfg = _plan(mask)

    isq = 1.0 / np.sqrt(np.float32(D))
    # fused QK matrix, scaled into e4m3 range
    M = (Wq.T @ Wk) * isq
    Mpad = np.zeros((DP, DP), np.float32)
    Mpad[:D, :D] = M * SC_M
    m8 = Mpad.astype(F8)
    # key-side score bias direction (bq . K_k term)
    u_k = (Wk.T @ bq) * isq
    wvp = np.zeros((DP, D), np.float32)
    wvp[:D, :D] = Wv.T * SC_V
    wv8 = wvp.astype(F8)

    W1e = W1 * ln_g[None, :]
    b1e = b1 + W1 @ ln_b
    w1s = np.ascontiguousarray((-W1e.sum(axis=1)).reshape(1, 512)).astype(BF16)
    w1 = np.ascontiguousarray(W1e.T).astype(BF16)
    b1p = np.ascontiguousarray(b1e.reshape(4, 128).T).astype(np.float32)
    w2 = np.ascontiguousarray(W2.T).astype(BF16)
    b2p = np.ascontiguousarray(b2.reshape(2, 128).T).astype(np.float32)
    w3 = np.ascontiguousarray(W3.T).astype(BF16)
    b3p = np.ascontiguousarray(b3.reshape(1, 128).T).astype(np.float32)
    w4 = np.ascontiguousarray(W4.T).astype(BF16)
    b4p = np.ascontiguousarray(b4.reshape(10, 1)).astype(np.float32)
    id4 = np.eye(4, dtype=np.float32)

    shared = dict(
        m8=m8, wv=wv8,
        w1s=w1s, w1=w1, w2=w2, w3=w3, w4=w4,
        b1=b1p, b2=b2p, b3=b3p, b4=b4p, id4=id4,
    )
    in_maps = []
    for core in range(NCORES):
        xt = np.zeros((BPC, DP, SPMAX), F8)
        xn = np.zeros((BPC, SPMAX, D), BF16)
        mnp = np.full((BPC, 128, 5), -1e9, np.float32)
        mfs = np.zeros((BPC, 128, 5), np.float32)
        for jj in range(BPC):
            b = perm[8 * jj + core]
            idx = np.nonzero(mask[b])[0]
            n = len(idx)
            assert n <= cfg[jj][0], f"sample {b}: {n} > slot budget {cfg[jj][0]}"
            xk = x[b, idx]                                # [n, D]
            xt[jj, :D, :n] = xk.T.astype(F8)
            xn[jj, :n] = (xk + bv[None, :]).astype(BF16)
            mnpj = np.full(SPMAX, -1e9, np.float32)
            mnpj[:n] = xk @ u_k
            mnp[jj] = mnpj.reshape(5, 128).T
            mfsj = np.zeros(SPMAX, np.float32)
            mfsj[:n] = 1.0 / n
            mfs[jj] = mfsj.reshape(5, 128).T
        m = dict(shared)
        m.update(xt=xt, xn=xn, mnp=mnp, mfs=mfs)
        in_maps.append(m)
    return in_maps, perm, cfg


def assemble(results, perm):
    """results: list of 8 dicts with 'out' [10, BPC] -> [32, 10] f32."""
    out = np.zeros((B, 10), np.float32)
    for core, r in enumerate(results):
        o = np.asarray(r["out"], np.float32)  # [10, BPC]
        for jj in range(BPC):
            out[perm[8 * jj + core]] = o[:, jj]
    return out


def kernel(**inputs):
    from concourse.bass_utils import run_bass_kernel_spmd

    in_maps, perm, cfg = host_prep(inputs)
    nc = _get_nc(cfg)
    res = run_bass_kernel_spmd(nc, in_maps, core_ids=list(range(NCORES)))
    out = assemble(res.results, perm)
    if not np.all(np.isfinite(out)):
        # rare transient device flake observed (~1 in 30 runs): retry once
        res = run_bass_kernel_spmd(nc, in_maps, core_ids=list(range(NCORES)))
        out = assemble(res.results, perm)
    return out


# revision 15
# speedup vs baseline: 1.2032x; 1.2032x over previous
"""Trainium2 Bass kernel for DeepProteinClassifier.

Contract: kernel(**inputs) takes the FULL unsharded inputs and returns
the FULL [32, 10] float32 output.

Sharding: data-parallel over batch B=32 across 8 NeuronCores (4 samples
per core); all weights replicated.

Optimizations over the naive formulation:
- Mask compaction + per-slot sizing: only mask==1 positions matter.
  Samples are sorted by kept-count and dealt into 4 slots of 8 (one per
  core), so slot j's compiled budget is the max count of its group:
  (SPQ, NK) per slot, e.g. (544,5),(520,5),(520,5),(504,4) -- the
  smallest slot runs last, shrinking both PE work and the endgame tail.
  The program is compiled per budget tuple (cached).
- Fused QK: scores = x M x^T + key-bias with M = Wq^T Wk / sqrt(D);
  per-query constants cancel in softmax; per-key term is a host bias
  folded into the exp bias. Deletes one 960x960 projection.
- fp8 (e4m3) matmuls in DoubleRow mode, fp32 PSUM accumulation.
- LayerNorm stats via the DVE bn_stats/bn_aggr hardware path (two
  equal 480-wide chunks so the aggregate combine is exact); residual
  adds fused into the PSUM drains (scalar_tensor_tensor); the second
  residual half runs on the otherwise-idle GPSIMD engine.
- Masked-mean pool as PE matvec with a zero-padded 4-column lhsT so
  sample j lands on PSUM partition j; all 4 samples accumulate into one
  persistent PSUM pair, drained once before the MLP transposes (no
  SBUF->SBUF DMAs).
- PE warm-up: dummy matmuls issued during the initial DMA wait flip the
  HAM clock gate to 2.4 GHz before real work; attention weights are
  fetched on the GpSimd queue so the ACT table load can't delay them.
"""

import numpy as np
import ml_dtypes

B, S, D = 32, 1024, 960
NCORES = 8
BPC = B // NCORES   # 4 samples (slots) per core
SPMAX = 640         # kept positions hard cap (5 tiles of 128)
DP = 1024           # padded contraction dim (8 chunks of 128, 4 DR pairs)
NDC = 8             # number of 128-row d chunks
PD = 120            # MLP-side partition size (960 = 8*120)
LN_EPS = 1e-5
SC_M = 1024.0       # host scale on M (undone in exp drain)
SC_V = 16.0         # host scale on Wv (undone in V drain)
BF16 = ml_dtypes.bfloat16
F8 = ml_dtypes.float8_e4m3

_CACHE = {}


def _build_nc(cfg):
    """cfg: tuple of (spq, nk) per slot j=0..3, spq multiple of 8."""
    import concourse.tile as tile
    from concourse import bacc, mybir

    class _Bacc(bacc.Bacc):
        """Bacc with the ACT table chooser steered to the combined
        ln+exp function set so LayerNorm's Ln/Exp pair and the softmax
        exp share ONE table (no ACT_TABLE_LOAD thrash)."""

        def insert_act_table_loads(self):
            import bass_rust as _bass_rust
            from concourse.hw_specs import get_activation_tables

            has_activation = any(
                isinstance(i, mybir.InstActivation)
                for b in self.main_func.blocks
                for i in b.instructions
            )
            if not has_activation:
                return
            tables = list(get_activation_tables(self.m.arch).items())
            combo = next(
                (f for n, f in tables if n == "natural_log_exp_and_others"), None
            )
            if combo is not None:
                tables = [
                    (n, f if n == "natural_log_exp_and_others" else f - combo)
                    for n, f in tables
                ]
            _bass_rust.insert_act_table_loads(self, tables)

    f32 = mybir.dt.float32
    bf16 = mybir.dt.bfloat16
    f8 = mybir.dt.float8e4
    Alu = mybir.AluOpType
    Act = mybir.ActivationFunctionType
    DR = mybir.MatmulPerfMode.DoubleRow

    nc = _Bacc("TRN2", target_bir_lowering=False, debug=False)

    # T1T tile width: covers the largest slot's q budget, 16B-aligned
    # (DoubleRow rhs strides must be multiples of 16)
    spq_alloc = (max(c[0] for c in cfg) + 15) // 16 * 16

    # ---- DRAM parameters (per-core shard) ----
    xt_h = nc.declare_dram_parameter("xt", [BPC, DP, SPMAX], f8, isOutput=False)
    xn_h = nc.declare_dram_parameter("xn", [BPC, SPMAX, D], bf16, isOutput=False)
    mnp_h = nc.declare_dram_parameter("mnp", [BPC, 128, 5], f32, isOutput=False)
    mfs_h = nc.declare_dram_parameter("mfs", [BPC, 128, 5], f32, isOutput=False)
    m8_h = nc.declare_dram_parameter("m8", [DP, DP], f8, isOutput=False)
    wv_h = nc.declare_dram_parameter("wv", [DP, D], f8, isOutput=False)
    w1s_h = nc.declare_dram_parameter("w1s", [1, 512], bf16, isOutput=False)
    w1_h = nc.declare_dram_parameter("w1", [D, 512], bf16, isOutput=False)
    w2_h = nc.declare_dram_parameter("w2", [512, 256], bf16, isOutput=False)
    w3_h = nc.declare_dram_parameter("w3", [256, 128], bf16, isOutput=False)
    w4_h = nc.declare_dram_parameter("w4", [128, 10], bf16, isOutput=False)
    b1_h = nc.declare_dram_parameter("b1", [128, 4], f32, isOutput=False)
    b2_h = nc.declare_dram_parameter("b2", [128, 2], f32, isOutput=False)
    b3_h = nc.declare_dram_parameter("b3", [128, 1], f32, isOutput=False)
    b4_h = nc.declare_dram_parameter("b4", [10, 1], f32, isOutput=False)
    id4_h = nc.declare_dram_parameter("id4", [4, 4], f32, isOutput=False)
    out_h = nc.declare_dram_parameter("out", [10, BPC], f32, isOutput=True)

    with tile.TileContext(nc) as tc:
        with (
            tc.tile_pool(name="wpool", bufs=1) as wpool,
            tc.tile_pool(name="xpool", bufs=3) as xpool,
            tc.tile_pool(name="big", bufs=2) as big,
            tc.tile_pool(name="stats", bufs=3) as stats,
            tc.tile_pool(name="psum", bufs=8, space="PSUM") as psum,
        ):
            # ---- PE warm-up: flip the HAM clock gate to 8/8 during the
            #      initial DMA wait (dummy matmuls on a memset scratch) --
            wscr = wpool.tile([128, 512], f8, name="wscr")
            nc.vector.memset(wscr[:], 0.25)
            psw = psum.tile([128, 512], f32, tag="mm", name="psw")
            for i in range(8):
                nc.tensor.matmul(
                    psw[:], lhsT=wscr[:, 0:128], rhs=wscr[:],
                    start=(i == 0), stop=(i == 7),
                )
            wsink = wpool.tile([1, 1], f32, name="wsink")
            nc.vector.tensor_copy(wsink[:], psw[0:1, 0:1])
            epsc = wpool.tile([128, 1], f32, name="epsc")
            nc.vector.memset(epsc[:], LN_EPS)

            def load_sample(j, defer=False):
                nk = cfg[j][1]
                kc = nk * 128
                xt_sb = xpool.tile([128, NDC, SPMAX], f8, tag="xt", name=f"xt{j}")
                if defer:
                    # pair-granular so the first T1T matmuls start early
                    for p in range(4):
                        nc.sync.dma_start(
                            xt_sb[:, 2 * p : 2 * p + 2, 0:kc],
                            xt_h[j, 256 * p : 256 * (p + 1), 0:kc].rearrange(
                                "(c p) s -> p c s", p=128
                            ),
                        )
                else:
                    nc.sync.dma_start(
                        xt_sb[:, :, 0:kc],
                        xt_h[j, :, 0:kc].rearrange("(c p) s -> p c s", p=128),
                    )
                xn_sb = xpool.tile([128, 5, D], bf16, tag="xn", name=f"xn{j}")
                mnp_sb = stats.tile([128, 5], f32, tag="mnp", name=f"mnp{j}")
                mfs_sb = stats.tile([128, 5], f32, tag="mfs", name=f"mfs{j}")
                if not defer:
                    nc.sync.dma_start(
                        xn_sb[:, 0:nk, :],
                        xn_h[j, 0:kc].rearrange("(t p) d -> p t d", p=128),
                    )
                    nc.sync.dma_start(mnp_sb[:], mnp_h[j])
                    nc.sync.dma_start(mfs_sb[:], mfs_h[j])
                return xt_sb, xn_sb, mnp_sb, mfs_sb

            # xt0 + attention weights first (they gate the first matmuls).
            # Weights go on the GpSimd HWDGE queue: the scalar queue's
            # ACT_TABLE_LOAD would delay their ring kick by ~1.3us.
            sample0 = load_sample(0, defer=True)
            m8_sb = wpool.tile([128, NDC, DP], f8)
            wv_sb = wpool.tile([128, NDC, DP], f8)
            nc.gpsimd.dma_start(
                m8_sb[:, :, 0:256],
                m8_h[:, 0:256].rearrange("(c p) n -> p c n", p=128),
            )
            nc.gpsimd.dma_start(
                m8_sb[:, :, 256:512],
                m8_h[:, 256:512].rearrange("(c p) n -> p c n", p=128),
            )
            nc.gpsimd.dma_start(
                wv_sb[:, :, 0:512],
                wv_h[:, 0:512].rearrange("(c p) n -> p c n", p=128),
            )
            nc.gpsimd.dma_start(
                m8_sb[:, :, 512:DP],
                m8_h[:, 512:DP].rearrange("(c p) n -> p c n", p=128),
            )
            nc.gpsimd.dma_start(
                wv_sb[:, :, 512:D],
                wv_h[:, 512:D].rearrange("(c p) n -> p c n", p=128),
            )
            nk0 = cfg[0][1]
            nc.sync.dma_start(
                sample0[1][:, 0:nk0, :],
                xn_h[0, 0 : nk0 * 128].rearrange("(t p) d -> p t d", p=128),
            )
            nc.sync.dma_start(sample0[2][:], mnp_h[0])
            nc.sync.dma_start(sample0[3][:], mfs_h[0])

            pooled_sb = wpool.tile([BPC, D + 1], f32, name="pooled_sb")
            nc.vector.memset(pooled_sb[:], 0.0)
            murow = wpool.tile([1, BPC], bf16)
            mlp_w = {}

            def load_mlp_weights():
                w1s_sb = wpool.tile([1, 512], bf16, name="w1s_sb")
                nc.sync.dma_start(w1s_sb[:], w1s_h[:])
                mlp_w["w1s"] = w1s_sb
                w1_sb = wpool.tile([PD, NDC, 512], bf16, name="w1_sb")
                nc.sync.dma_start(w1_sb[:], w1_h[:].rearrange("(c p) n -> p c n", p=PD))
                w2_sb = wpool.tile([128, 4, 256], bf16, name="w2_sb")
                nc.sync.dma_start(w2_sb[:], w2_h[:].rearrange("(c p) n -> p c n", p=128))
                w3_sb = wpool.tile([128, 2, 128], bf16, name="w3_sb")
                nc.sync.dma_start(w3_sb[:], w3_h[:].rearrange("(c p) n -> p c n", p=128))
                w4_sb = wpool.tile([128, 10], bf16, name="w4_sb")
                nc.sync.dma_start(w4_sb[:], w4_h[:])
                b1_sb = wpool.tile([128, 4], f32, name="b1_sb")
                nc.sync.dma_start(b1_sb[:], b1_h[:])
                b2_sb = wpool.tile([128, 2], f32, name="b2_sb")
                nc.sync.dma_start(b2_sb[:], b2_h[:])
                b3_sb = wpool.tile([128, 1], f32, name="b3_sb")
                nc.sync.dma_start(b3_sb[:], b3_h[:])
                b4_sb = wpool.tile([10, 1], f32, name="b4_sb")
                nc.sync.dma_start(b4_sb[:], b4_h[:])
                id4_sb = wpool.tile([4, 4], f32, name="id4_sb")
                nc.sync.dma_start(id4_sb[:], id4_h[:])
                mlp_w.update(w1=w1_sb, w2=w2_sb, w3=w3_sb, w4=w4_sb,
                             b1=b1_sb, b2=b2_sb, b3=b3_sb, b4=b4_sb, id4=id4_sb)

            pending_pool = None

            for j in range(BPC):
                spq, nk = cfg[j]
                w0 = min(512, spq)      # main q stream width
                tw = spq - w0           # tail q width (may be 0)
                # q-tile widths (partial last tile)
                qws = [min(128, spq - qt * 128) for qt in range(nk)]
                qws = [w for w in qws if w > 0]
                qt_n = len(qws)

                if j == 0:
                    xt_sb, xn_sb, mnp_sb, mfs_sb = sample0
                else:
                    xt_sb, xn_sb, mnp_sb, mfs_sb = load_sample(j)
                if j == 1:
                    load_mlp_weights()

                # ---- T1T = M^T-chunks @ xT-chunks: [do(1024), q(spq)] fp8 --
                T1T = big.tile([128, NDC, spq_alloc], f8, tag="T1T", name=f"T1T{j}")
                V = big.tile([128, 5, 1024], f8, tag="V", name=f"V{j}")
                nc.vector.memset(V[:, 0:nk, D : D + 1], 1.0)

                def t1_chunks(ts, te):
                  with nc.named_scope(f"s{j}_t1"):
                    for t in range(ts, te):
                        psA = psum.tile([128, w0], f32, tag="mm", name="pt1a")
                        if tw:
                            psB = psum.tile([128, tw], f32, tag="mm", name="pt1b")
                        for p in range(4):
                            lw = m8_sb[:, 2 * p : 2 * p + 2, t * 128 : (t + 1) * 128]
                            nc.tensor.matmul(
                                psA[:], lhsT=lw,
                                rhs=xt_sb[:, 2 * p : 2 * p + 2, 0:w0],
                                start=(p == 0), stop=(p == 3), perf_mode=DR,
                            )
                            if tw:
                                nc.tensor.matmul(
                                    psB[:], lhsT=lw,
                                    rhs=xt_sb[:, 2 * p : 2 * p + 2, 512:spq],
                                    start=(p == 0), stop=(p == 3), perf_mode=DR,
                                )
                        nc.vector.tensor_copy(T1T[:, t, 0:w0], psA[:])
                        if tw:
                            nc.vector.tensor_copy(T1T[:, t, 512:spq], psB[:])

                def v_half(lo, hi):
                  with nc.named_scope(f"s{j}_v"):
                    for st in range(nk):
                        ps = psum.tile([128, 512], f32, tag="mm", name="psv")
                        for p in range(4):
                            lx = xt_sb[:, 2 * p : 2 * p + 2, st * 128 : (st + 1) * 128]
                            nc.tensor.matmul(
                                ps[:, 0 : hi - lo], lhsT=lx,
                                rhs=wv_sb[:, 2 * p : 2 * p + 2, lo:hi],
                                start=(p == 0), stop=(p == 3), perf_mode=DR,
                            )
                        if lo == 0:
                            nc.scalar.activation(
                                V[:, st, lo:hi], ps[:, 0 : hi - lo],
                                Act.Copy, scale=1.0 / SC_V,
                            )
                        else:
                            nc.vector.tensor_scalar_mul(
                                V[:, st, lo:hi], ps[:, 0 : hi - lo], 1.0 / SC_V
                            )

                # T1T t0-3 needs only the first m8 half; V's first half then
                # runs while the later weight-DMA halves land
                t1_chunks(0, 4)
                v_half(0, 512)
                t1_chunks(4, NDC)
                v_half(512, D)

                # ---- ST = xT^T @ T1T; ET = exp(ST/1024 + keybias) fp8 ----
                ET = big.tile([128, 5, SPMAX], f8, tag="ET", name=f"ET{j}")
                with nc.named_scope(f"s{j}_st"):
                    for kt in range(nk):
                        psA = psum.tile([128, w0], f32, tag="mm", name="pssa")
                        if tw:
                            psB = psum.tile([128, tw], f32, tag="mm", name="pssb")
                        for p in range(4):
                            lx = xt_sb[:, 2 * p : 2 * p + 2, kt * 128 : (kt + 1) * 128]
                            nc.tensor.matmul(
                                psA[:], lhsT=lx,
                                rhs=T1T[:, 2 * p : 2 * p + 2, 0:w0],
                                start=(p == 0), stop=(p == 3), perf_mode=DR,
                            )
                            if tw:
                                nc.tensor.matmul(
                                    psB[:], lhsT=lx,
                                    rhs=T1T[:, 2 * p : 2 * p + 2, 512:spq],
                                    start=(p == 0), stop=(p == 3), perf_mode=DR,
                                )
                        nc.scalar.activation(
                            ET[:, kt, 0:w0], psA[:], Act.Exp,
                            bias=mnp_sb[:, kt : kt + 1], scale=1.0 / SC_M,
                        )
                        if tw:
                            nc.scalar.activation(
                                ET[:, kt, 512:spq], psB[:], Act.Exp,
                                bias=mnp_sb[:, kt : kt + 1], scale=1.0 / SC_M,
                            )

                # previous sample's pool matvec lands here: its AL/H are long
                # ready, and it fills the PE while the ET exp drains finish
                if pending_pool is not None:
                    pending_pool()
                    pending_pool = None

                # ---- context + residual + per-tile LN stats --------------
                # LayerNorm is per-row: tile qt's alpha is ready as soon as
                # its context drains, so the pool matvec pipelines per-tile.
                H = big.tile([128, 5, 1024], bf16, tag="H", name=f"H{j}")
                recips = stats.tile([128, 5], f32, tag="recips", name=f"rc{j}")
                bn = stats.tile([128, 5, 12], f32, tag="bn", name=f"bn{j}")
                MV = stats.tile([128, 5, 2], f32, tag="MV", name=f"MV{j}")
                lnv = stats.tile([128, 5], f32, tag="lnv", name=f"lnv{j}")
                rs = stats.tile([128, 5], f32, tag="rs", name=f"rs{j}")
                AL4 = stats.tile([128, 5, BPC], bf16, tag="AL4", name=f"AL{j}")
                nc.vector.memset(AL4[:], 0.0)
                with nc.named_scope(f"s{j}_ctx"):
                    for qt in range(qt_n):
                        pw = qws[qt]
                        qo = qt * 128
                        ps0 = psum.tile([128, 512], f32, tag="mm", name="psc0")
                        ps1 = psum.tile([128, 449], f32, tag="mm", name="psc1")
                        for p in range(nk // 2):
                            le = ET[:, 2 * p : 2 * p + 2, qo : qo + pw]
                            nc.tensor.matmul(
                                ps0[:pw, :], lhsT=le,
                                rhs=V[:, 2 * p : 2 * p + 2, 0:512],
                                start=(p == 0), stop=(nk % 2 == 0 and p == nk // 2 - 1),
                                perf_mode=DR,
                            )
                            nc.tensor.matmul(
                                ps1[:pw, :], lhsT=le,
                                rhs=V[:, 2 * p : 2 * p + 2, 512 : D + 1],
                                start=(p == 0), stop=(nk % 2 == 0 and p == nk // 2 - 1),
                                perf_mode=DR,
                            )
                        if nk % 2:
                            le = ET[:, nk - 1, qo : qo + pw]
                            nc.tensor.matmul(
                                ps0[:pw, :], lhsT=le, rhs=V[:, nk - 1, 0:512],
                                start=False, stop=True,
                            )
                            nc.tensor.matmul(
                                ps1[:pw, :], lhsT=le, rhs=V[:, nk - 1, 512 : D + 1],
                                start=False, stop=True,
                            )
                        q = slice(qt, qt + 1)
                        # col 448 of ps1: softmax denom (V ones column)
                        nc.vector.reciprocal(recips[:pw, q], ps1[:pw, 448:449])
                        # H = ctx/r + xn, fused PSUM drain + residual
                        nc.vector.scalar_tensor_tensor(
                            H[:pw, qt, 0:512], ps0[:pw, :], recips[:pw, q],
                            xn_sb[:pw, qt, 0:512], Alu.mult, Alu.add,
                        )
                        cscr = stats.tile([128, 448], bf16, tag="cscr",
                                          name=f"cs{j}_{qt}", bufs=2)
                        nc.scalar.activation(
                            cscr[:pw, :], ps1[:pw, 0:448], Act.Copy,
                            scale=recips[:pw, q],
                        )
                        nc.gpsimd.tensor_add(
                            H[:pw, qt, 512:D], cscr[:pw, :],
                            xn_sb[:pw, qt, 512:D],
                        )
                        # LN stats via bn_stats/bn_aggr (2 equal 480 chunks)
                        nc.vector.bn_stats(bn[:pw, qt, 0:6], H[:pw, qt, 0:480])
                        nc.vector.bn_stats(bn[:pw, qt, 6:12], H[:pw, qt, 480:D])
                        nc.vector.bn_aggr(MV[:pw, qt, :], bn[:pw, qt, :])
                        # mean -> H col 960 (pooled into murow later)
                        nc.vector.tensor_copy(
                            H[:pw, qt, D : D + 1], MV[:pw, qt, 0:1]
                        )
                        # rs = exp(-0.5 ln(var+eps)); AL = mfs * rs
                        nc.scalar.activation(
                            lnv[:pw, q], MV[:pw, qt, 1:2], Act.Ln,
                            bias=epsc[:pw, :],
                        )
                        nc.scalar.activation(
                            rs[:pw, q], lnv[:pw, q], Act.Exp, scale=-0.5
                        )
                        nc.vector.tensor_tensor(
                            AL4[:pw, qt, j : j + 1], mfs_sb[:pw, q],
                            rs[:pw, q], Alu.mult,
                        )

                # ---- masked-mean pool as PE matvec; sample j's AL sits in
                #      lhsT column j so its row lands on PSUM partition j,
                #      making the drain into pooled_sb row j lane-aligned
                #      (no SBUF->SBUF DMA). Chunk c waits only on its own AL
                #      column so it pipelines with the LN chain.
                def emit_pool(j=j, AL4=AL4, H=H, qws=qws):
                    pq0 = psum.tile([BPC, 512], f32, tag="mm", name=f"pq0_{j}")
                    pq1 = psum.tile([BPC, 449], f32, tag="mm", name=f"pq1_{j}")
                    for c, pw in enumerate(qws):
                        nc.tensor.matmul(
                            pq0[:, :],
                            lhsT=AL4[:pw, c, :],
                            rhs=H[:pw, c, 0:512],
                            start=(c == 0), stop=(c == len(qws) - 1),
                        )
                        nc.tensor.matmul(
                            pq1[:, :],
                            lhsT=AL4[:pw, c, :],
                            rhs=H[:pw, c, 512 : D + 1],
                            start=(c == 0), stop=(c == len(qws) - 1),
                        )
                    # rows != j hold zeros (zero lhsT columns), so an
                    # in-place add deposits row j without touching others
                    nc.vector.tensor_add(
                        pooled_sb[:, 0:512], pq0[:, :], pooled_sb[:, 0:512]
                    )
                    nc.vector.tensor_add(
                        pooled_sb[:, 512 : D + 1], pq1[:, :],
                        pooled_sb[:, 512 : D + 1],
                    )

                if j == BPC - 1:
                    # last sample: no next-sample matmuls to hide behind --
                    # emit inline so pool chunks interleave with the LN chain
                    emit_pool()
                else:
                    pending_pool = emit_pool

            # ---- transposes (mu correction is folded into the W1 matmul
            #      as a rank-1 term, see w1s) ----
            pooledT = stats.tile([PD, NDC, BPC], bf16, tag="pT")
            for c in range(NDC):
                pst = psum.tile([128, 512], f32, tag="mm", name=f"pst{c}")
                nc.tensor.transpose(
                    pst[:PD, :BPC],
                    pooled_sb[:, c * PD : (c + 1) * PD],
                    mlp_w["id4"][:],
                )
                nc.scalar.activation(pooledT[:, c, :], pst[:PD, :BPC], Act.Copy)
            psmu = psum.tile([128, 512], f32, tag="mm", name="psmu")
            nc.tensor.transpose(psmu[:1, :BPC], pooled_sb[:, D : D + 1], mlp_w["id4"][:])
            nc.scalar.activation(murow[:, :], psmu[:1, :BPC], Act.Copy)

            # ---- MLP in transposed layout ----
            h1T = stats.tile([128, 4, BPC], bf16, tag="h1T")
            for m in range(4):
                ps = psum.tile([128, 512], f32, tag="mm", name=f"psm1{m}")
                for c in range(NDC):
                    nc.tensor.matmul(
                        ps[:, :BPC],
                        lhsT=mlp_w["w1"][:, c, m * 128 : (m + 1) * 128],
                        rhs=pooledT[:, c, :],
                        start=(c == 0), stop=False,
                    )
                # rank-1 mu correction: h1 += (-W1e @ ones) * mu
                nc.tensor.matmul(
                    ps[:, :BPC],
                    lhsT=mlp_w["w1s"][:, m * 128 : (m + 1) * 128],
                    rhs=murow[:, :],
                    start=False, stop=True,
                )
                nc.scalar.activation(
                    h1T[:, m, :], ps[:, :BPC], Act.Relu, bias=mlp_w["b1"][:, m : m + 1]
                )
            h2T = stats.tile([128, 2, BPC], bf16, tag="h2T")
            for m in range(2):
                ps = psum.tile([128, 512], f32, tag="mm", name=f"psm2{m}")
                for c in range(4):
                    nc.tensor.matmul(
                        ps[:, :BPC],
                        lhsT=mlp_w["w2"][:, c, m * 128 : (m + 1) * 128],
                        rhs=h1T[:, c, :],
                        start=(c == 0), stop=(c == 3),
                    )
                nc.scalar.activation(
                    h2T[:, m, :], ps[:, :BPC], Act.Relu, bias=mlp_w["b2"][:, m : m + 1]
                )
            h3T = stats.tile([128, 1, BPC], bf16, tag="h3T")
            ps = psum.tile([128, 512], f32, tag="mm", name="psm3")
            for c in range(2):
                nc.tensor.matmul(
                    ps[:, :BPC],
                    lhsT=mlp_w["w3"][:, c, :],
                    rhs=h2T[:, c, :],
                    start=(c == 0), stop=(c == 1),
                )
            nc.scalar.activation(
                h3T[:, 0, :], ps[:, :BPC], Act.Relu, bias=mlp_w["b3"][:, 0:1]
            )
            ps4 = psum.tile([128, 512], f32, tag="mm", name="psm4")
            nc.tensor.matmul(
                ps4[:10, :BPC], lhsT=mlp_w["w4"][:, :], rhs=h3T[:, 0, :],
                start=True, stop=True,
            )
            osb = stats.tile([10, BPC], f32, tag="osb")
            nc.scalar.activation(osb[:], ps4[:10, :BPC], Act.Identity, bias=mlp_w["b4"][:])
            nc.sync.dma_start(out_h[:], osb[:])

    nc.compile()
    return nc


def _get_nc(cfg):
    if cfg not in _CACHE:
        _CACHE[cfg] = _build_nc(cfg)
    return _CACHE[cfg]


def _plan(mask):
    """Sort samples by kept-count (desc), deal into 4 slots of 8 cores.

    Returns (perm, cfg): perm[8*j + c] = original sample index assigned
    to core c slot j; cfg[j] = (spq, nk) compile-time budget of slot j.
    """
    n = mask.sum(axis=1).astype(np.int64)
    perm = np.argsort(-n, kind="stable")
    cfg = []
    for j in range(BPC):
        nmax = int(n[perm[8 * j]])
        nmax = max(nmax, 8)
        assert nmax <= SPMAX, f"slot {j}: {nmax} kept positions > {SPMAX}"
        nk = (nmax + 127) // 128
        spq = min((nmax + 7) // 8 * 8, nk * 128)
        cfg.append((spq, nk))
    return perm, tuple(cfg)


def host_prep(inputs):
    """Build the 8 per-core in_maps from the full inputs."""
    x = np.asarray(inputs["x"], np.float32)
    mask = np.asarray(inputs["mask"])
    Wq, bq = np.asarray(inputs["Wq"], np.float32), np.asarray(inputs["bq"], np.float32)
    Wk = np.asarray(inputs["Wk"], np.float32)
    Wv, bv = np.asarray(inputs["Wv"], np.float32), np.asarray(inputs["bv"], np.float32)
    ln_g, ln_b = np.asarray(inputs["ln_g"], np.float32), np.asarray(inputs["ln_b"], np.float32)
    W1, b1 = np.asarray(inputs["W1"], np.float32), np.asarray(inputs["b1"], np.float32)
    W2, b2 = np.asarray(inputs["W2"], np.float32), np.asarray(inputs["b2"], np.float32)
    W3, b3 = np.asarray(inputs["W3"], np.float32), np.asarray(inputs["b3"], np.float32)
    W4, b4 = np.asarray(inputs["W4"], np.float32), np.asarray(inputs["b4"], np.float32)

    perm, cfg = _plan(mask)

    isq = 1.0 / np.sqrt(np.float32(D))
    # fused QK matrix, scaled into e4m3 range
    M = (Wq.T @ Wk) * isq
    Mpad = np.zeros((DP, DP), np.float32)
    Mpad[:D, :D] = M * SC_M
    m8 = Mpad.astype(F8)
    # key-side score bias direction (bq . K_k term)
    u_k = (Wk.T @ bq) * isq
    wvp = np.zeros((DP, D), np.float32)
    wvp[:D, :D] = Wv.T * SC_V
    wv8 = wvp.astype(F8)

    W1e = W1 * ln_g[None, :]
    b1e = b1 + W1 @ ln_b
    w1s = np.ascontiguousarray((-W1e.sum(axis=1)).reshape(1, 512)).astype(BF16)
    w1 = np.ascontiguousarray(W1e.T).astype(BF16)
    b1p = np.ascontiguousarray(b1e.reshape(4, 128).T).astype(np.float32)
    w2 = np.ascontiguousarray(W2.T).astype(BF16)
    b2p = np.ascontiguousarray(b2.reshape(2, 128).T).astype(np.float32)
    w3 = np.ascontiguousarray(W3.T).astype(BF16)
    b3p = np.ascontiguousarray(b3.reshape(1, 128).T).astype(np.float32)
    w4 = np.ascontiguousarray(W4.T).astype(BF16)
    b4p = np.ascontiguousarray(b4.reshape(10, 1)).astype(np.float32)
    id4 = np.eye(4, dtype=np.float32)

    shared = dict(
        m8=m8, wv=wv8,
        w1s=w1s, w1=w1, w2=w2, w3=w3, w4=w4,
        b1=b1p, b2=b2p, b3=b3p, b4=b4p, id4=id4,
    )
    in_maps = []
    for core in range(NCORES):
        xt = np.zeros((BPC, DP, SPMAX), F8)
        xn = np.zeros((BPC, SPMAX, D), BF16)
        mnp = np.full((BPC, 128, 5), -1e9, np.float32)
        mfs = np.zeros((BPC, 128, 5), np.float32)
        for jj in range(BPC):
            b = perm[8 * jj + core]
            idx = np.nonzero(mask[b])[0]
            n = len(idx)
            assert n <= cfg[jj][0], f"sample {b}: {n} > slot budget {cfg[jj][0]}"
            xk = x[b, idx]                                # [n, D]
            xt[jj, :D, :n] = xk.T.astype(F8)
            xn[jj, :n] = (xk + bv[None, :]).astype(BF16)
            mnpj = np.full(SPMAX, -1e9, np.float32)
            mnpj[:n] = xk @ u_k
            mnp[jj] = mnpj.reshape(5, 128).T
            mfsj = np.zeros(SPMAX, np.float32)
            mfsj[:n] = 1.0 / n
            mfs[jj] = mfsj.reshape(5, 128).T
        m = dict(shared)
        m.update(xt=xt, xn=xn, mnp=mnp, mfs=mfs)
        in_maps.append(m)
    return in_maps, perm, cfg


def assemble(results, perm):
    """results: list of 8 dicts with 'out' [10, BPC] -> [32, 10] f32."""
    out = np.zeros((B, 10), np.float32)
    for core, r in enumerate(results):
        o = np.asarray(r["out"], np.float32)  # [10, BPC]
        for jj in range(BPC):
            out[perm[8 * jj + core]] = o[:, jj]
    return out


def kernel(**inputs):
    from concourse.bass_utils import run_bass_kernel_spmd

    in_maps, perm, cfg = host_prep(inputs)
    nc = _get_nc(cfg)
    res = run_bass_kernel_spmd(nc, in_maps, core_ids=list(range(NCORES)))
    out = assemble(res.results, perm)
    if not np.all(np.isfinite(out)):
        # rare transient device flake observed (~1 in 30 runs): retry once
        res = run_bass_kernel_spmd(nc, in_maps, core_ids=list(range(NCORES)))
        out = assemble(res.results, perm)
    return out
